# revision 12
# baseline (speedup 1.0000x reference)
"""Trainium2 Bass kernel for nn_EnergyLoss (acoustic energy MSE loss).

Math (per batch row b, focal point m):
    A = amp * exp(j*phase)                          # [B, 64] complex
    E[b,m] = C1*|h[m,:]@A[b]|^2 + C2*sum_d |q[m,d,:]@A[b]|^2
    loss = mean((1e5*E - true)^2)

Real-arithmetic formulation used here:
    X = [amp*cos(phase) | amp*sin(phase)]           # [B, 128]
    Y = X @ W                                       # [B, 512]
      W packs re/im of (scaled) h and q0..q2 so that
    pred[b,m] = sum_{s in h-blocks} Y[b, s*64+m]^2 - sum_{s in q-blocks} Y[b, s*64+m]^2
    loss = sum((pred - true)^2) / (B*M)

Sharding: pure data parallel over batch across 8 cores; h/q (as W) replicated;
host sums the per-core partial sums.
"""

import os
from contextlib import ExitStack

import numpy as np

import concourse.bacc as bacc
import concourse.bass as bass
import concourse.mybir as mybir
import concourse.tile as tile
from concourse import bass_utils

N_CORES = 8
B = 32768
BC = B // N_CORES  # 4096 rows per core
N = 64
M = 64
NT = BC // 128  # 32 tiles of 128 rows per core
GT = 4          # tiles per epilogue group
NG = NT // GT   # 8 groups
PI = float(np.pi)

F32 = mybir.dt.float32
AF = mybir.ActivationFunctionType
ALU = mybir.AluOpType

_CACHE: dict = {}


def _build_w(h: np.ndarray, q: np.ndarray) -> np.ndarray:
    """Pack [128, 512] f32 weight matrix.

    Column blocks (each 64 wide, one per focal point m):
      0: h_re, 1: h_im            scaled by sqrt(1e5*C1)       (positive sign)
      2,3: q0_re, q0_im; 4,5: q1_re, q1_im; 6,7: q2_re, q2_im  scaled by
         sqrt(1e5*|C2|)                                        (negative sign)
    Rows: 0..63 multiply Ar (= amp*cos), 64..127 multiply Ai (= amp*sin).
    For complex C (rows of h or q[d]):
      re[b,m] = sum_n Ar*Cr - Ai*Ci ;  im[b,m] = sum_n Ar*Ci + Ai*Cr
    """
    R = 0.000865
    CONST = 2 * np.pi * R**3
    C_SOUND = 343.0
    RHO = 1.225
    C1 = CONST * (1 / (6 * RHO * C_SOUND**2))
    C2 = CONST * (-RHO / 4)
    s1 = np.sqrt(1e5 * C1)
    s2 = np.sqrt(1e5 * abs(C2))

    w = np.zeros((128, 512), dtype=np.float64)
    mats = [(h * s1, 0)] + [(q[:, d, :] * s2, 2 + 2 * d) for d in range(3)]
    for cmat, blk in mats:
        cr = np.real(cmat)  # [m, n]
        ci = np.imag(cmat)
        # re block at columns [blk*64, (blk+1)*64): W[n, m] = cr[m, n]; W[64+n, m] = -ci[m, n]
        w[0:64, blk * 64:(blk + 1) * 64] = cr.T
        w[64:128, blk * 64:(blk + 1) * 64] = -ci.T
        # im block: W[n, m] = ci[m, n]; W[64+n, m] = cr[m, n]
        w[0:64, (blk + 1) * 64:(blk + 2) * 64] = ci.T
        w[64:128, (blk + 1) * 64:(blk + 2) * 64] = cr.T
    return w.astype(np.float32)


def _build_w2(h: np.ndarray, q: np.ndarray):
    """Rank-2 re-factorization of the per-m quadratic forms.

    M_m = sum_s sigma_s w_{s,m} w_{s,m}^T (exact rank 8); keep the top-2
    |eigenvalue| directions (measured end-to-end loss rel-err ~6e-6).
    Returns (W2 [128, 128] f32 with cols j = i*64+m scaled by sqrt|lambda|,
    signs [64, 2]).
    """
    w8 = _build_w(h, q).astype(np.float64)
    wv = w8.reshape(128, 8, 64)
    sig = np.array([1, 1, -1, -1, -1, -1, -1, -1], dtype=np.float64)
    w2 = np.zeros((128, 128))
    signs = np.zeros((64, 2))
    for m in range(64):
        vs = wv[:, :, m]
        mm = (vs * sig) @ vs.T
        ev, u = np.linalg.eigh(mm)
        idx = np.argsort(-np.abs(ev))[:2]
        lam = ev[idx]
        signs[m] = np.sign(lam)
        for i in range(2):
            w2[:, i * 64 + m] = u[:, idx[i]] * np.sqrt(abs(lam[i]))
    return w2.astype(np.float32), signs.astype(np.float32)


def _build_module():
    nc = bacc.Bacc(
        "TRN2",
        target_bir_lowering=False,
        debug=False,
        enable_asserts=False,
        num_devices=N_CORES,
    )
    amp_d = nc.dram_tensor("amp", [BC, N], F32, kind="ExternalInput")
    phase_d = nc.dram_tensor("phase", [BC, N], F32, kind="ExternalInput")
    true_d = nc.dram_tensor("true_e", [BC, M], F32, kind="ExternalInput")
    w_d = nc.dram_tensor("w", [128, 512], F32, kind="ExternalInput")
    ident_d = nc.dram_tensor("ident", [128, 128], F32, kind="ExternalInput")
    out_d = nc.dram_tensor("partials", [128, NG], F32, kind="ExternalOutput")

    with tile.TileContext(nc) as tc, ExitStack() as ctx:
        const = ctx.enter_context(tc.tile_pool(name="const", bufs=1))
        big = ctx.enter_context(tc.tile_pool(name="big", bufs=1))
        work = ctx.enter_context(tc.tile_pool(name="work", bufs=3))
        epi = ctx.enter_context(tc.tile_pool(name="epi", bufs=2))
        pp = ctx.enter_context(tc.tile_pool(name="pp", bufs=2, space="PSUM"))
        ppy = ctx.enter_context(tc.tile_pool(name="ppy", bufs=1, space="PSUM"))

        w_s = const.tile([128, 512], F32)
        nc.sync.dma_start(w_s[:], w_d.ap())
        ident_s = const.tile([128, 128], F32)
        nc.sync.dma_start(ident_s[:], ident_d.ap())

        # DRAM [(p t) n] -> SBUF [p, (t n)]: partition p holds rows p*NT..p*NT+NT-1
        amp_s = big.tile([128, NT * N], F32)
        nc.sync.dma_start(amp_s[:], amp_d.ap().rearrange("(p t) n -> p (t n)", p=128))
        phase_s = big.tile([128, NT * N], F32)
        nc.sync.dma_start(
            phase_s[:], phase_d.ap().rearrange("(p t) n -> p (t n)", p=128)
        )
        true_s = big.tile([128, NT * M], F32)
        nc.sync.dma_start(
            true_s[:], true_d.ap().rearrange("(p t) n -> p (t n)", p=128)
        )

        # ACT Sin domain is [-pi, pi]; phase is [0, 2pi).
        # sin path: Sin(phase - pi) = -sin(phase); the sign is absorbed by
        #   negating rows 64:128 of W on the host (X carries -amp*sin).
        # cos path: wrap(phase + pi/2) into [-pi, pi] on DVE, then Sin.
        neg_pi = const.tile([128, 1], F32)
        nc.gpsimd.memset(neg_pi[:], -PI)
        zero_b = const.tile([128, 1], F32)
        nc.gpsimd.memset(zero_b[:], 0.0)
        sin_s = big.tile([128, NT * N], F32)
        nc.scalar.activation(sin_s[:], phase_s[:], AF.Sin, bias=neg_pi[:])
        phw = big.tile([128, NT * N], F32)
        nc.vector.add_range_wrap(
            phw[:], phase_s[:], shift=PI / 2, bound=PI, period=2 * PI
        )
        cos_s = big.tile([128, NT * N], F32)
        nc.scalar.activation(cos_s[:], phw[:], AF.Sin, bias=zero_b[:])

        # X[p, t, 0:64] = amp*cos, X[p, t, 64:128] = amp*sin
        x_buf = big.tile([128, NT, 128], F32)
        amp_v = amp_s[:].rearrange("p (t n) -> p t n", t=NT)
        nc.vector.tensor_tensor(
            x_buf[:, :, 0:64], amp_v, cos_s[:].rearrange("p (t n) -> p t n", t=NT),
            op=ALU.mult,
        )
        nc.vector.tensor_tensor(
            x_buf[:, :, 64:128], amp_v, sin_s[:].rearrange("p (t n) -> p t n", t=NT),
            op=ALU.mult,
        )

        part = const.tile([128, NG], F32)

        true_v = true_s[:].rearrange("p (t m) -> p t m", t=NT)
        for g in range(NG):
            y_ps = ppy.tile([128, GT, 512], F32)
            for i in range(GT):
                t = g * GT + i
                xt_ps = pp.tile([128, 128], F32)
                nc.tensor.transpose(xt_ps[:], x_buf[:, t, :], ident_s[:])
                xt_sb = work.tile([128, 128], F32)
                nc.scalar.copy(xt_sb[:], xt_ps[:])
                nc.tensor.matmul(
                    y_ps[:, i, :], lhsT=xt_sb[:], rhs=w_s[:], start=True, stop=True
                )
            # squares (ACT: single-input op, PSUM source allowed)
            sq = epi.tile([128, GT, 512], F32)
            nc.scalar.activation(sq[:], y_ps[:], AF.Square)
            # block reduction: h part (s=0,1) minus q part (s=2..7)
            sq_v = sq[:].rearrange("p t (s m) -> p t m s", s=8)
            r_h = epi.tile([128, GT, 64], F32)
            nc.vector.tensor_reduce(
                r_h[:], sq_v[:, :, :, 0:2], axis=mybir.AxisListType.X, op=ALU.add
            )
            r_q = epi.tile([128, GT, 64], F32)
            nc.vector.tensor_reduce(
                r_q[:], sq_v[:, :, :, 2:8], axis=mybir.AxisListType.X, op=ALU.add
            )
            d1 = epi.tile([128, GT, 64], F32)
            nc.vector.tensor_tensor(d1[:], r_h[:], r_q[:], op=ALU.subtract)
            diff = epi.tile([128, GT, 64], F32)
            nc.vector.tensor_tensor(
                diff[:], d1[:], true_v[:, g * GT:(g + 1) * GT, :], op=ALU.subtract
            )
            scrap = epi.tile([128, GT, 64], F32)
            nc.vector.scalar_tensor_tensor(
                out=scrap[:],
                in0=diff[:],
                scalar=1.0,
                in1=diff[:],
                op0=ALU.mult,
                op1=ALU.mult,
                accum_out=part[:, g:g + 1],
            )

        nc.sync.dma_start(out_d.ap(), part[:])

    nc.compile()
    return nc


def _build_module_v2():
    """Optimized pipeline (rank-2 operator, bf16 matmuls, transposed layout).

    Per 128-row tile t (32 per core), all within TileContext:
      X_t = [amp*cos | -amp*sin] bf16  (sign folded into W2 rows 64:128)
      XT_t = PE-transpose(X_t) -> PSUM -> SBUF     (k on partitions)
      Y^T_g = W2^T @ XT_g   [128 j, 512 b] f32 PSUM   (j = i*64+m, i<2)
      SQ_g = ACT Square(Y^T_g) -> bf16 SBUF
      D[par*64+m, b] += Sel^T @ SQ  (sel signs), D -= true^T (PE transpose
         with -I identity, accumulating in PSUM)
      loss partial = ACT Square(D) with accum_out (per-partition sums)
    """
    BF16 = mybir.dt.bfloat16
    nc = bacc.Bacc(
        "TRN2",
        target_bir_lowering=False,
        debug=False,
        enable_asserts=False,
        num_devices=N_CORES,
    )
    amp_d = nc.dram_tensor("amp", [BC, N], F32, kind="ExternalInput")
    phase_d = nc.dram_tensor("phase", [BC, N], F32, kind="ExternalInput")
    true_d = nc.dram_tensor("true_e", [BC, M], F32, kind="ExternalInput")
    w2_d = nc.dram_tensor("w2", [128, 128], BF16, kind="ExternalInput")
    sel_d = nc.dram_tensor("sel", [128, 64], BF16, kind="ExternalInput")
    identb_d = nc.dram_tensor("identb", [128, 128], BF16, kind="ExternalInput")
    negi_d = nc.dram_tensor("negi", [128, 128], F32, kind="ExternalInput")
    NB = NT // 8  # D banks (8 tiles per bank)
    out_d = nc.dram_tensor("partials", [128, NB], F32, kind="ExternalOutput")

    with tile.TileContext(nc) as tc, ExitStack() as ctx:
        const = ctx.enter_context(tc.tile_pool(name="const", bufs=1))
        big = ctx.enter_context(tc.tile_pool(name="big", bufs=1))
        sqp = ctx.enter_context(tc.tile_pool(name="sqp", bufs=3))
        xtp = ctx.enter_context(tc.tile_pool(name="xtp", bufs=3))
        scrapp = ctx.enter_context(tc.tile_pool(name="scrapp", bufs=2))
        ppx = ctx.enter_context(tc.tile_pool(name="ppx", bufs=2, space="PSUM"))
        ppy = ctx.enter_context(tc.tile_pool(name="ppy", bufs=2, space="PSUM"))
        ppd = ctx.enter_context(tc.tile_pool(name="ppd", bufs=2, space="PSUM"))

        w2_s = const.tile([128, 128], BF16)
        nc.sync.dma_start(w2_s[:], w2_d.ap())
        sel_s = const.tile([128, 64], BF16)
        nc.sync.dma_start(sel_s[:], sel_d.ap())
        identb_s = const.tile([128, 128], BF16)
        nc.sync.dma_start(identb_s[:], identb_d.ap())
        negi_s = const.tile([128, 128], F32)
        nc.sync.dma_start(negi_s[:], negi_d.ap())

        amp_s = big.tile([128, NT * N], F32)
        nc.sync.dma_start(amp_s[:], amp_d.ap().rearrange("(p t) n -> p (t n)", p=128))
        phase_s = big.tile([128, NT * N], F32)
        nc.sync.dma_start(
            phase_s[:], phase_d.ap().rearrange("(p t) n -> p (t n)", p=128)
        )
        true_s = big.tile([128, NT * M], F32)
        nc.sync.dma_start(
            true_s[:], true_d.ap().rearrange("(p t) n -> p (t n)", p=128)
        )

        neg_pi = const.tile([128, 1], F32)
        nc.gpsimd.memset(neg_pi[:], -PI)
        zero_b = const.tile([128, 1], F32)
        nc.gpsimd.memset(zero_b[:], 0.0)

        # sin path: Sin(phase - pi) = -sin(phase); sign folded into W2.
        sin_s = big.tile([128, NT * N], F32)
        nc.scalar.activation(sin_s[:], phase_s[:], AF.Sin, bias=neg_pi[:])
        # cos path: wrap(phase + pi/2) into [-pi, pi], then Sin.
        phw = big.tile([128, NT * N], F32)
        nc.vector.add_range_wrap(
            phw[:], phase_s[:], shift=PI / 2, bound=PI, period=2 * PI
        )
        cos_s = big.tile([128, NT * N], F32)
        nc.scalar.activation(cos_s[:], phw[:], AF.Sin, bias=zero_b[:])

        x_buf = big.tile([128, NT, 128], BF16)
        amp_v = amp_s[:].rearrange("p (t n) -> p t n", t=NT)
        nc.vector.tensor_tensor(
            x_buf[:, :, 0:64], amp_v, cos_s[:].rearrange("p (t n) -> p t n", t=NT),
            op=ALU.mult,
        )
        nc.vector.tensor_tensor(
            x_buf[:, :, 64:128], amp_v, sin_s[:].rearrange("p (t n) -> p t n", t=NT),
            op=ALU.mult,
        )

        part = const.tile([128, NB], F32)
        true_v = true_s[:].rearrange("p (t m) -> p t m", t=NT)

        for k in range(NB):  # D bank: 8 tiles = 2 groups of 4
            d_ps = ppd.tile([128, 512], F32)
            # -true^T for the 4 tile-pairs of this bank (each [128,2,64] -> [128,128])
            for pr in range(4):
                tp = 8 * k + 2 * pr
                nc.tensor.matmul(
                    d_ps[:, pr * 128:(pr + 1) * 128],
                    lhsT=true_v[:, tp:tp + 2, :],
                    rhs=negi_s[:],
                    is_transpose=True,
                    start=(pr == 0),
                    stop=False,
                    skip_group_check=True,
                )
            for gi in range(2):  # groups within bank
                g = 2 * k + gi
                xt_ps = ppx.tile([128, 512], BF16)
                for i in range(GT):
                    t = g * GT + i
                    nc.tensor.transpose(
                        xt_ps[:, i * 128:(i + 1) * 128], x_buf[:, t, :], identb_s[:]
                    )
                xt_sb = xtp.tile([128, 512], BF16)
                nc.vector.tensor_copy(xt_sb[:], xt_ps[:])
                y_ps = ppy.tile([128, 512], F32)
                nc.tensor.matmul(
                    y_ps[:], lhsT=w2_s[:], rhs=xt_sb[:], start=True, stop=True
                )
                sq = sqp.tile([128, 512], BF16)
                nc.scalar.activation(sq[:], y_ps[:], AF.Square)
                for i in range(GT):
                    t = g * GT + i
                    par = t % 2
                    pr_in_bank = (t - 8 * k) // 2
                    last = gi == 1 and i == GT - 1
                    nc.tensor.matmul(
                        d_ps[par * 64:(par + 1) * 64,
                             pr_in_bank * 128:(pr_in_bank + 1) * 128],
                        lhsT=sel_s[:],
                        rhs=sq[:, i * 128:(i + 1) * 128],
                        start=False,
                        stop=last,
                        skip_group_check=True,
                    )
            scrap = scrapp.tile([128, 512], BF16)
            nc.scalar.activation(
                scrap[:], d_ps[:], AF.Square, accum_out=part[:, k:k + 1]
            )

        nc.sync.dma_start(out_d.ap(), part[:])

    nc.compile()
    return nc


def kernel(pred_amplitude, pred_phase, true_energy, h, q):
    import ml_dtypes

    amp = np.ascontiguousarray(np.asarray(pred_amplitude, dtype=np.float32))
    phase = np.ascontiguousarray(np.asarray(pred_phase, dtype=np.float32))
    true = np.ascontiguousarray(np.asarray(true_energy, dtype=np.float32))

    version = int(os.environ.get("KERNEL_VERSION", "2"))
    if version == 1:
        w = _build_w(np.asarray(h), np.asarray(q))
        # device computes -amp*sin(phase) in X rows 64:128; negate the
        # matching W rows so X@W is unchanged
        w[64:128, :] *= -1.0
        ident = np.eye(128, dtype=np.float32)
        if "nc1" not in _CACHE:
            _CACHE["nc1"] = _build_module()
        nc = _CACHE["nc1"]
        extra = {"w": w, "ident": ident}
    else:
        w2, signs = _build_w2(np.asarray(h), np.asarray(q))
        w2 = w2.copy()
        w2[64:128, :] *= -1.0  # device X carries -amp*sin in rows 64:128
        sel = np.zeros((128, 64), dtype=np.float32)
        for m in range(64):
            for i in range(2):
                # D accumulates true^T (+I transpose) minus pred^T, so Sel
                # carries negated signs; (true-pred)^2 == (pred-true)^2.
                sel[i * 64 + m, m] = -signs[m, i]
        identb = np.eye(128, dtype=ml_dtypes.bfloat16)
        negi = np.eye(128, dtype=np.float32)
        if "nc2" not in _CACHE:
            _CACHE["nc2"] = _build_module_v2()
        nc = _CACHE["nc2"]
        extra = {
            "w2": w2.astype(ml_dtypes.bfloat16),
            "sel": sel.astype(ml_dtypes.bfloat16),
            "identb": identb,
            "negi": negi,
        }

    in_maps = []
    for c in range(N_CORES):
        sl = slice(c * BC, (c + 1) * BC)
        in_maps.append(
            {"amp": amp[sl], "phase": phase[sl], "true_e": true[sl], **extra}
        )

    res = bass_utils.run_bass_kernel_spmd(
        nc, in_maps, core_ids=list(range(N_CORES)),
        trace=bool(int(os.environ.get("KERNEL_TRACE", "0"))),
    )
    total = np.float64(0.0)
    for r in res.results:
        total += r["partials"].astype(np.float64).sum()
    loss = np.float32(total / (B * M))
    if bool(int(os.environ.get("KERNEL_TRACE", "0"))):
        _CACHE["last_exec_time_ns"] = res.exec_time_ns
    return np.asarray(loss, dtype=np.float32)
